# revision 14
# baseline (speedup 1.0000x reference)
"""CenterLoss kernel for Trainium2 (8 NeuronCores, data-parallel over batch).

loss = mean_i( ||nx_i||^2 + ||c_{l_i}||^2 - 2*nx_i.c_{l_i} )
     = mean_i( ||nx_i - c_{l_i}||^2 ),  nx_i = x_i / max(||x_i||, EPS)

The reference's (batch, num_classes) distmat is masked down to one column
per row, so only a gather of centers[labels] is needed (memory regime).

Sharding: batch 16384 -> 8 cores x 2048 rows, centers replicated; row
p*16+j of a core's shard lives at SBUF partition p, free block j. The
gather is 16 INDIRECT1D ops (HW consumes one offset per dest partition
row; ~1.4us/op is the SWDGE issue rate and dominates the kernel). The
x-side pipeline (square/rowsum/sqrt/max/recip/normalize) runs under the
gathers on ACT+DVE; d = nx - c and the Square+accumulate reduction are
chunked per gather block so only the last tiny chunk trails. Raw bacc
(no Tile) with manual semaphores to minimize fixed prologue/epilogue.
Each core returns 5 per-partition partial sums; the host combines.
"""

import numpy as np

B, C, D = 16384, 8192, 64
N_CORES = 8
ROWS = B // N_CORES
P = 128
J = ROWS // P            # 16
F = J * D
BLOCKS = [6, 6, 2, 1, 1]
NBLK = len(BLOCKS)
BSTART = [sum(BLOCKS[:b]) for b in range(NBLK)]
EPS = 1e-12

_CACHE = {}


def _build():
    from contextlib import ExitStack

    import concourse.bass as bass
    from concourse import bacc, mybir

    nc = bacc.Bacc("TRN2", target_bir_lowering=False, debug=False,
                   num_devices=N_CORES, dynamic_dma_scratch_size=65536)
    f32 = mybir.dt.float32
    x = nc.dram_tensor("x", [ROWS, D], f32, kind="ExternalInput").ap()
    labels = nc.dram_tensor("labels", [P, J], mybir.dt.int32,
                            kind="ExternalInput").ap()
    centers = nc.dram_tensor("centers", [C, D], f32,
                             kind="ExternalInput").ap()
    out = nc.dram_tensor("out", [P, NBLK], f32, kind="ExternalOutput").ap()

    with ExitStack() as ctx:
        def sb(n, s, dt=f32):
            return ctx.enter_context(nc.sbuf_tensor(n, s, dt))
        lab_t = sb("lab_t", [P, J], mybir.dt.int32)
        x_t = sb("x_t", [P, F])
        c_t = sb("c_t", [P, F])
        xx = sb("xx", [P, F])
        sx = sb("sx", [P, J])
        mn = sb("mn", [P, J])
        inv = sb("inv", [P, J])
        nx = sb("nx", [P, F])
        acc = sb("acc", [P, NBLK])
        L = ctx.enter_context(nc.semaphore("Lsem"))
        X = ctx.enter_context(nc.semaphore("Xsem"))
        G = [ctx.enter_context(nc.semaphore(f"G{i}")) for i in range(NBLK)]
        A = ctx.enter_context(nc.semaphore("Asem"))   # ACT-produced events
        V = ctx.enter_context(nc.semaphore("Vsem"))   # DVE-produced events

        # ---- Sync: labels in, result out ----
        nc.sync.dma_start(lab_t[:], labels[:]).then_inc(L, 16)
        nc.sync.wait_ge(A, 2 + NBLK)
        nc.sync.dma_start(out, acc[:]).then_inc(L, 16)
        nc.sync.wait_ge(L, 32)

        # ---- GpSimd: 16 gathers back to back ----
        nc.gpsimd.wait_ge(L, 16)
        for j in range(J):
            b = next(i for i in range(NBLK)
                     if BSTART[i] <= j < BSTART[i] + BLOCKS[i])
            nc.gpsimd.indirect_dma_start(
                out=c_t[:, j * D:(j + 1) * D],
                out_offset=None,
                in_=centers[:],
                in_offset=bass.IndirectOffsetOnAxis(ap=lab_t[:, j:j + 1],
                                                    axis=0),
            ).then_inc(G[b], 16)

        # ---- Scalar/ACT: x in on its HWDGE ring, squares ----
        # A events: 1=xx, 2=mn(sqrt), 2+b+1 = chunk b accumulated
        nc.scalar.dma_start(x_t[:], x.rearrange("(p j) d -> p (j d)", p=P)
                            ).then_inc(X, 16)
        nc.scalar.wait_ge(X, 16)
        nc.scalar.square(xx[:], x_t[:]).then_inc(A, 1)
        nc.scalar.wait_ge(V, 1)
        nc.scalar.sqrt(mn[:], sx[:]).then_inc(A, 1)
        for b in range(NBLK):
            fb, f0 = BLOCKS[b] * D, BSTART[b] * D
            nc.scalar.wait_ge(V, 5 + b)
            nc.scalar.activation(c_t[:, f0:f0 + fb], c_t[:, f0:f0 + fb],
                                 mybir.ActivationFunctionType.Square,
                                 accum_out=acc[:, b:b + 1]).then_inc(A, 1)

        # ---- Vector/DVE ----
        # V events: 1=sx, 2=mn(max), 3=inv, 4=nx, 4+b+1 = chunk b sub done
        nc.vector.wait_ge(A, 1)
        nc.vector.reduce_sum(sx[:], xx[:].rearrange("p (j d) -> p j d", d=D),
                             axis=mybir.AxisListType.X).then_inc(V, 1)
        nc.vector.wait_ge(A, 2)
        nc.vector.tensor_scalar_max(mn[:], mn[:], EPS).then_inc(V, 1)
        nc.vector.wait_ge(V, 2)
        nc.vector.reciprocal(inv[:], mn[:]).then_inc(V, 1)
        nc.vector.wait_ge(V, 3)
        iap = inv[:]
        inv_bc = bass.AP(tensor=iap.tensor, offset=iap.offset,
                         ap=list(iap.ap) + [[0, D]])
        nc.vector.tensor_tensor(
            out=nx[:].rearrange("p (j d) -> p j d", d=D),
            in0=x_t[:].rearrange("p (j d) -> p j d", d=D),
            in1=inv_bc,
            op=mybir.AluOpType.mult,
        ).then_inc(V, 1)
        nc.vector.wait_ge(V, 4)
        for b in range(NBLK):
            fb, f0 = BLOCKS[b] * D, BSTART[b] * D
            nc.vector.wait_ge(G[b], 16 * BLOCKS[b])
            nc.vector.tensor_sub(c_t[:, f0:f0 + fb], nx[:, f0:f0 + fb],
                                 c_t[:, f0:f0 + fb]).then_inc(V, 1)

    nc.compile()
    return nc


def _get_nc():
    if "nc" not in _CACHE:
        _CACHE["nc"] = _build()
    return _CACHE["nc"]


def _run(x, labels, centers, trace=False):
    from concourse.bass_utils import run_bass_kernel_spmd

    x = np.ascontiguousarray(np.asarray(x, dtype=np.float32))
    labels = np.asarray(labels).astype(np.int32)
    centers = np.ascontiguousarray(np.asarray(centers, dtype=np.float32))

    in_maps = []
    for i in range(N_CORES):
        in_maps.append({
            "x": x[i * ROWS:(i + 1) * ROWS],
            "labels": np.ascontiguousarray(
                labels[i * ROWS:(i + 1) * ROWS].reshape(P, J)),
            "centers": centers,
        })
    res = run_bass_kernel_spmd(_get_nc(), in_maps,
                               core_ids=list(range(N_CORES)), trace=trace)
    total = np.float64(0.0)
    for r in res.results:
        total += np.float64(r["out"].sum(dtype=np.float64))
    loss = np.array(np.float32(total / B))
    return loss, res


def kernel(x, labels, centers):
    loss, _ = _run(x, labels, centers, trace=False)
    return loss
